# revision 5
# baseline (speedup 1.0000x reference)
"""Contrastive loss (SimCLR-style) TRN2 Bass kernel, 8-core data-parallel, fp8.

Math: z [8192, 256] f32 ->
  zn = z / ||z||row ; S = (zn @ zn.T)/0.1 ; diag masked; row log_softmax;
  loss = -mean_i( S[i, pos(i)] - logsumexp_j S[i, j] ), pos(i) = (i+4096) % 8192.

Strategy: shard rows across 8 cores (1024 rows each). Each core's input zf is
ROTATED on the host (np.roll) so its own row block sits at rows/cols [0, 1024)
-- one uniform SPMD program, no per-core offsets. The core normalizes all of
rotated z in pipelined column groups (squares on Pool, reduce/rsqrt/scale on
DVE; rsqrt = Quake bit-trick + 3 Newton steps so ACT keeps its Exp table),
bounces zn to DRAM, X-bar DMA-transposes back as bf16 [128, 2, cols], then
quantizes to fp8e4 (cancellation makes fp8 noise average out: measured 6e-7
end-to-end). The similarity block is computed with fp8 DoubleRow matmuls
(K=256 contracted in one instruction, 2 fp8 weights/PE cell), drained by one
fused ACT Exp(10x-10) + row-accumulate per (m-tile, group). d_pos and d_ii
are the diagonals of fixed sub-blocks of the PSUM logits (cols [mt*128,..)
and [4096+mt*128,..)), extracted with an eye-mask multiply + row reduce, so
no partner block or separate dot products are needed. Device outputs per-row
d_pos and the self-corrected rowsum; the host computes
loss = -mean(10*d_pos - 10 - log(rowsum)).
"""

import numpy as np

N = 8192
K = 256
N_CORES = 8
BLK = N // N_CORES          # 1024 rows per core
MT = BLK // 128             # 8 m-tiles per core
NT = N // 128               # 64 row tiles of full z
GROUP_TILES = [8, 4, 4, 16, 16, 16]  # row tiles per pipeline group (sums to NT)
NGROUP = len(GROUP_TILES)
POS_G = 4                   # group whose columns start at 4096 (pos diagonal)
TEMP_INV = 10.0             # 1/temperature
QMAGIC = 0x5F3759DF

_CACHE = {}


def _build():
    import concourse.bass as bass
    import concourse.tile as tile
    from concourse import bacc, mybir
    from concourse.bass_interp import get_hw_module

    F32, BF16, F16 = mybir.dt.float32, mybir.dt.bfloat16, mybir.dt.float16
    FP8 = mybir.dt.float8e4
    I32 = mybir.dt.int32
    AF, ALU = mybir.ActivationFunctionType, mybir.AluOpType
    AX = mybir.AxisListType
    PM = mybir.MatmulPerfMode

    nc = bacc.Bacc("TRN2", target_bir_lowering=False, debug=False,
                   enable_asserts=False, num_devices=N_CORES)

    zf_in = nc.dram_tensor("zf", [N, K], F32, kind="ExternalInput").ap()
    eye_in = nc.dram_tensor("eye", [128, 128], F32, kind="ExternalInput").ap()
    dpos_out = nc.dram_tensor("dpos", [128, MT], F32, kind="ExternalOutput").ap()
    rs_out = nc.dram_tensor("rs", [128, MT], F32, kind="ExternalOutput").ap()

    with tile.TileContext(nc) as tc:
        with (
            tc.tile_pool(name="big", bufs=1) as big,
            tc.tile_pool(name="pipe", bufs=3) as pipe,
            tc.tile_pool(name="work", bufs=2) as work,
            tc.tile_pool(name="stat", bufs=1) as stat,
            tc.tile_pool(name="dram", bufs=1, space=bass.MemorySpace.DRAM) as dram,
            tc.tile_pool(name="ps", bufs=2, space=bass.MemorySpace.PSUM) as psp,
        ):
            magic = stat.tile([128, 16], I32)
            nc.vector.memset(magic[:], QMAGIC)
            bias_m10 = stat.tile([128, 1], F32)
            nc.vector.memset(bias_m10[:], -TEMP_INV)
            eye = stat.tile([128, 128], F32)
            nc.sync.dma_start(eye[:], eye_in)

            znT_own = big.tile([128, 2, BLK], FP8)   # lhsT: own cols [0, 1024)
            rs_part = stat.tile([128, MT, NGROUP], F32)
            d_pos = stat.tile([128, MT], F32)
            d_ii = stat.tile([128, MT], F32)

            def rsqrt_dve(ss, nt, tag):
                """rsq = 1/sqrt(ss), Quake init + 3 Newton iterations (DVE)."""
                ssi = ss[:].bitcast(I32)
                sh = work.tile([128, nt], I32, tag="sh")
                nc.vector.tensor_scalar(sh[:], ssi, 1, None,
                                        op0=ALU.arith_shift_right)
                y = stat.tile([128, nt], F32, tag=f"y_{tag}")
                yi = y[:].bitcast(I32)
                nc.vector.tensor_sub(yi, magic[:, 0:nt], sh[:])
                for it in range(3):
                    y2 = work.tile([128, nt], F32, tag="nwt")
                    nc.vector.tensor_mul(y2[:], y[:], y[:])
                    xy2 = work.tile([128, nt], F32, tag="nwt")
                    nc.vector.tensor_mul(xy2[:], ss[:], y2[:])
                    c = work.tile([128, nt], F32, tag="nwt")
                    nc.vector.tensor_scalar(c[:], xy2[:], -0.5, 1.5,
                                            op0=ALU.mult, op1=ALU.add)
                    yn = stat.tile([128, nt], F32, tag=f"y{it}_{tag}")
                    nc.vector.tensor_mul(yn[:], y[:], c[:])
                    y = yn
                return y

            # ---- preload all group row tiles (f32 -> bf16 on DMA) ----
            g_zbf = []
            c0 = 0
            for g, tpg in enumerate(GROUP_TILES):
                zt = pipe.tile([128, tpg, K], BF16, tag="zbf")
                nc.gpsimd.dma_start(
                    zt[:], zf_in[c0:c0 + tpg * 128, :].rearrange(
                        "(t p) k -> p t k", p=128))
                g_zbf.append(zt)
                c0 += tpg * 128

            def extract_diag(ps, off, dst, mt):
                tmp = work.tile([128, 128], F32, tag="dg")
                nc.vector.tensor_mul(tmp[:], ps[:, off:off + 128], eye[:])
                nc.vector.reduce_sum(dst[:, mt:mt + 1], tmp[:], axis=AX.X)

            # ---- main pipeline over column groups ----
            col0 = 0
            for g, tpg in enumerate(GROUP_TILES):
                gw = tpg * 128
                zbf = g_zbf[g]
                # normalize: squares on Pool, reduce + rsqrt on DVE
                ss = stat.tile([128, tpg], F32, tag=f"ss{g}")
                for h0 in range(0, tpg, 4):
                    h1 = min(h0 + 4, tpg)
                    sq = work.tile([128, h1 - h0, K], F16, tag="sq")
                    nc.gpsimd.tensor_mul(sq[:], zbf[:, h0:h1, :],
                                         zbf[:, h0:h1, :])
                    nc.vector.reduce_sum(ss[:, h0:h1], sq[:], axis=AX.X)
                rsq = rsqrt_dve(ss, tpg, f"g{g}")
                zn = pipe.tile([128, tpg, K], BF16, tag="zn")
                for t in range(tpg):
                    eng = nc.vector if t % 2 == 0 else nc.gpsimd
                    eng.tensor_scalar(zn[:, t, :], zbf[:, t, :],
                                      rsq[:, t:t + 1], None, op0=ALU.mult)
                # bounce to DRAM, transpose back, quantize to fp8
                zn_dram = dram.tile([tpg * 128, K], BF16, tag=f"znd{g}")
                nc.sync.dma_start(zn_dram.rearrange("(t p) k -> p t k", p=128),
                                  zn[:])
                znT_bf = pipe.tile([128, 2, gw], BF16, tag="znTb")
                nc.sync.dma_start(znT_bf[:, 0, :], zn_dram[:, 0:128],
                                  transpose=True)
                nc.sync.dma_start(znT_bf[:, 1, :], zn_dram[:, 128:256],
                                  transpose=True)
                if g == 0:
                    rhs = znT_own[:]
                else:
                    znTg = pipe.tile([128, 2, gw], FP8, tag="znT8")
                    rhs = znTg[:]
                for j in range(2):
                    eng = nc.vector if j == 0 else nc.gpsimd
                    eng.tensor_scalar(rhs[:, j, :], znT_bf[:, j, :], 1.0, None,
                                      op0=ALU.mult)
                # fp8 DoubleRow matmuls + fused exp/accumulate
                for mt in range(MT):
                    ps = psp.tile([128, gw], F32, tag="ps")
                    for s0 in range(0, gw, 512):
                        nc.tensor.matmul(ps[:, s0:s0 + 512],
                                         znT_own[:, :, mt * 128:(mt + 1) * 128],
                                         rhs[:, :, s0:s0 + 512],
                                         start=True, stop=True,
                                         perf_mode=PM.DoubleRow)
                    if g == 0:
                        extract_diag(ps, mt * 128, d_ii, mt)
                    elif g == POS_G:
                        extract_diag(ps, mt * 128, d_pos, mt)
                    expo = work.tile([128, gw], BF16, tag="expo")
                    nc.scalar.activation(expo[:], ps[:], AF.Exp,
                                         bias=bias_m10[:], scale=TEMP_INV,
                                         accum_out=rs_part[:, mt, g:g + 1])
                col0 += gw

            # ---- epilogue: rowsum minus self term ----
            rs_sum = stat.tile([128, MT], F32)
            nc.vector.reduce_sum(rs_sum[:], rs_part[:], axis=AX.X)
            self_t = stat.tile([128, MT], F32)
            nc.scalar.activation(self_t[:], d_ii[:], AF.Exp,
                                 bias=bias_m10[:], scale=TEMP_INV)
            rs_corr = stat.tile([128, MT], F32)
            nc.vector.tensor_sub(rs_corr[:], rs_sum[:], self_t[:])

            nc.sync.dma_start(dpos_out, d_pos[:])
            nc.sync.dma_start(rs_out, rs_corr[:])

    nc.compile()
    nc.m = get_hw_module(nc.m)
    return nc


def _get_nc():
    if "nc" not in _CACHE:
        _CACHE["nc"] = _build()
    return _CACHE["nc"]


def _in_maps(z):
    z = np.ascontiguousarray(z, dtype=np.float32)
    eye = np.eye(128, dtype=np.float32)
    maps = []
    for c in range(N_CORES):
        maps.append({
            "zf": np.roll(z, -c * BLK, axis=0),
            "eye": eye,
        })
    return maps


def _finish(results):
    total = 0.0
    for c in range(N_CORES):
        dpos = results[c]["dpos"].astype(np.float64)
        rs = results[c]["rs"].astype(np.float64)
        total += (TEMP_INV * dpos - TEMP_INV - np.log(rs)).sum()
    return np.float32(-total / N)


def kernel(z):
    from concourse import bass_utils
    nc = _get_nc()
    res = bass_utils.run_bass_kernel_spmd(nc, _in_maps(z),
                                          core_ids=list(range(N_CORES)))
    return _finish(res.results)


# revision 8
# speedup vs baseline: 1.4571x; 1.4571x over previous
"""Contrastive loss (SimCLR-style) TRN2 Bass kernel, 8-core data-parallel, fp8.

Math: z [8192, 256] f32 ->
  zn = z / ||z||row ; S = (zn @ zn.T)/0.1 ; diag masked; row log_softmax;
  loss = -mean_i( S[i, pos(i)] - logsumexp_j S[i, j] ), pos(i) = (i+4096) % 8192.

Strategy: shard rows across 8 cores (1024 rows each). Each core's input zf is
ROTATED on the host (np.roll) so its own row block sits at rows/cols [0, 1024)
-- one uniform SPMD program, no per-core offsets. Per 1024-column group: load
rows f32->bf16 (SWDGE cast DMA), normalize on DVE (squares fused with the row
reduce via scalar_tensor_tensor accum_out; rsqrt = Quake bit-trick + 2 Newton
steps so ACT keeps its Exp table loaded), PE-transpose zn into PSUM (16
[128,128] bf16 transposes -- no DRAM bounce), quantize-copy PSUM -> SBUF fp8e4
on DVE/Pool, then fp8 DoubleRow matmuls (K=256 contracted per instruction, 2
fp8 weights/PE cell) drained by one fused ACT Exp(10x-10) + row-accumulate per
m-tile. fp8 quantization noise averages out over rows (measured ~1e-6
end-to-end). d_pos and d_ii are diagonals of fixed sub-blocks of the PSUM
logits, extracted with one fused eye-mask multiply-accumulate each. Device
outputs per-row d_pos and the self-corrected rowsum; the host computes
loss = -mean(10*d_pos - 10 - log(rowsum)).
"""

import numpy as np

N = 8192
K = 256
N_CORES = 8
BLK = N // N_CORES          # 1024 rows per core
MT = BLK // 128             # 8 m-tiles per core
NT = N // 128               # 64 row tiles of full z
TPG = 8                     # row tiles per pipeline group
NGROUP = NT // TPG          # 8 groups of 1024 columns
GW = TPG * 128              # 1024
POS_G = 4                   # group covering cols [4096, 5120): pos diagonal
TEMP_INV = 10.0             # 1/temperature
QMAGIC = 0x5F3759DF

_CACHE = {}


def _build():
    import concourse.bass as bass
    import concourse.tile as tile
    from concourse import bacc, mybir
    from concourse.bass_interp import get_hw_module

    F32, BF16, F16 = mybir.dt.float32, mybir.dt.bfloat16, mybir.dt.float16
    FP8 = mybir.dt.float8e4
    I32 = mybir.dt.int32
    AF, ALU = mybir.ActivationFunctionType, mybir.AluOpType
    AX = mybir.AxisListType
    PM = mybir.MatmulPerfMode

    nc = bacc.Bacc("TRN2", target_bir_lowering=False, debug=False,
                   enable_asserts=False, num_devices=N_CORES)

    zf_in = nc.dram_tensor("zf", [N, K], F32, kind="ExternalInput").ap()
    eye_in = nc.dram_tensor("eye", [128, 128], F32, kind="ExternalInput").ap()
    eyeb_in = nc.dram_tensor("eyeb", [128, 128], BF16, kind="ExternalInput").ap()
    dpos_out = nc.dram_tensor("dpos", [128, MT], F32, kind="ExternalOutput").ap()
    rs_out = nc.dram_tensor("rs", [128, MT], F32, kind="ExternalOutput").ap()

    with tile.TileContext(nc) as tc:
        with (
            tc.tile_pool(name="big", bufs=1) as big,
            tc.tile_pool(name="pipe", bufs=3) as pipe,
            tc.tile_pool(name="work", bufs=2) as work,
            tc.tile_pool(name="stat", bufs=1) as stat,
            tc.tile_pool(name="ps", bufs=2, space=bass.MemorySpace.PSUM) as psp,
            tc.tile_pool(name="pt", bufs=2, space=bass.MemorySpace.PSUM) as ptp,
        ):
            magic = stat.tile([128, TPG], I32)
            nc.vector.memset(magic[:], QMAGIC)
            bias_m10 = stat.tile([128, 1], F32)
            nc.vector.memset(bias_m10[:], -TEMP_INV)
            eye = stat.tile([128, 128], F32)
            nc.sync.dma_start(eye[:], eye_in)
            eyeb = stat.tile([128, 128], BF16)
            nc.sync.dma_start(eyeb[:], eyeb_in)

            znT_own = big.tile([128, 2, BLK], FP8)   # lhsT: own cols [0, 1024)
            rs_part = stat.tile([128, MT, NGROUP], F32)
            d_pos = stat.tile([128, MT], F32)
            d_ii = stat.tile([128, MT], F32)

            def rsqrt_dve(ss, tag):
                """rsq = 1/sqrt(ss), Quake init + 2 Newton iterations (DVE)."""
                ssi = ss[:].bitcast(I32)
                sh = work.tile([128, TPG], I32, tag="sh")
                nc.vector.tensor_scalar(sh[:], ssi, 1, None,
                                        op0=ALU.arith_shift_right)
                y = stat.tile([128, TPG], F32, tag=f"y_{tag}")
                yi = y[:].bitcast(I32)
                nc.vector.tensor_sub(yi, magic[:], sh[:])
                for it in range(2):
                    y2 = work.tile([128, TPG], F32, tag="nwt")
                    nc.vector.tensor_mul(y2[:], y[:], y[:])
                    xy2 = work.tile([128, TPG], F32, tag="nwt")
                    nc.vector.tensor_mul(xy2[:], ss[:], y2[:])
                    c = work.tile([128, TPG], F32, tag="nwt")
                    nc.vector.tensor_scalar(c[:], xy2[:], -0.5, 1.5,
                                            op0=ALU.mult, op1=ALU.add)
                    yn = stat.tile([128, TPG], F32, tag=f"y{it}_{tag}")
                    nc.vector.tensor_mul(yn[:], y[:], c[:])
                    y = yn
                return y

            # ---- preload all group row tiles (f32 -> bf16 cast on SWDGE) ----
            g_zbf = []
            for g in range(NGROUP):
                zt = pipe.tile([128, TPG, K], BF16, tag="zbf")
                nc.gpsimd.dma_start(
                    zt[:], zf_in[g * GW:(g + 1) * GW, :].rearrange(
                        "(t p) k -> p t k", p=128))
                g_zbf.append(zt)

            def extract_diag(ps, off, dst, mt):
                tmp = work.tile([128, 128], F32, tag="dg")
                nc.vector.scalar_tensor_tensor(
                    tmp[:], ps[:, off:off + 128], 1.0, eye[:],
                    op0=ALU.mult, op1=ALU.mult,
                    accum_out=dst[:, mt:mt + 1])

            # ---- main pipeline over column groups ----
            for g in range(NGROUP):
                zbf = g_zbf[g]
                # normalize: squares fused with row-reduce, then rsqrt, scale
                ss = stat.tile([128, TPG], F32, tag=f"ss{g}")
                for t in range(TPG):
                    sq = work.tile([128, K], F16, tag="sq")
                    nc.vector.scalar_tensor_tensor(
                        sq[:], zbf[:, t, :], 1.0, zbf[:, t, :],
                        op0=ALU.mult, op1=ALU.mult,
                        accum_out=ss[:, t:t + 1])
                rsq = rsqrt_dve(ss, f"g{g}")
                zn = pipe.tile([128, TPG, K], BF16, tag="zn")
                for t in range(TPG):
                    nc.vector.tensor_scalar(zn[:, t, :], zbf[:, t, :],
                                            rsq[:, t:t + 1], None, op0=ALU.mult)
                # PE-transpose zn into PSUM (bf16), quantize-copy to SBUF fp8
                pt = ptp.tile([128, 2, GW], BF16, tag="pt")
                for t in range(TPG):
                    for j in range(2):
                        nc.tensor.transpose(
                            pt[:, j, t * 128:(t + 1) * 128],
                            zn[:, t, j * 128:(j + 1) * 128], eyeb[:])
                if g == 0:
                    rhs = znT_own[:]
                else:
                    znTg = pipe.tile([128, 2, GW], FP8, tag="znT8")
                    rhs = znTg[:]
                nc.vector.tensor_scalar(rhs[:, 0, :], pt[:, 0, :], 1.0, None,
                                        op0=ALU.mult)
                nc.vector.tensor_scalar(rhs[:, 1, :], pt[:, 1, :], 1.0, None,
                                        op0=ALU.mult)
                # fp8 DoubleRow matmuls + fused exp/accumulate
                for mt in range(MT):
                    ps = psp.tile([128, GW], F32, tag="ps")
                    for s0 in range(0, GW, 512):
                        nc.tensor.matmul(ps[:, s0:s0 + 512],
                                         znT_own[:, :, mt * 128:(mt + 1) * 128],
                                         rhs[:, :, s0:s0 + 512],
                                         start=True, stop=True,
                                         perf_mode=PM.DoubleRow)
                    if g == 0:
                        extract_diag(ps, mt * 128, d_ii, mt)
                    elif g == POS_G:
                        extract_diag(ps, mt * 128, d_pos, mt)
                    expo = work.tile([128, GW], BF16, tag="expo")
                    nc.scalar.activation(expo[:], ps[:], AF.Exp,
                                         bias=bias_m10[:], scale=TEMP_INV,
                                         accum_out=rs_part[:, mt, g:g + 1])

            # ---- epilogue: rowsum minus self term ----
            rs_sum = stat.tile([128, MT], F32)
            nc.vector.reduce_sum(rs_sum[:], rs_part[:], axis=AX.X)
            self_t = stat.tile([128, MT], F32)
            nc.scalar.activation(self_t[:], d_ii[:], AF.Exp,
                                 bias=bias_m10[:], scale=TEMP_INV)
            rs_corr = stat.tile([128, MT], F32)
            nc.vector.tensor_sub(rs_corr[:], rs_sum[:], self_t[:])

            nc.sync.dma_start(dpos_out, d_pos[:])
            nc.sync.dma_start(rs_out, rs_corr[:])

    nc.compile()
    nc.m = get_hw_module(nc.m)
    return nc


def _get_nc():
    if "nc" not in _CACHE:
        _CACHE["nc"] = _build()
    return _CACHE["nc"]


def _in_maps(z):
    import ml_dtypes
    z = np.ascontiguousarray(z, dtype=np.float32)
    eye = np.eye(128, dtype=np.float32)
    eyeb = np.eye(128, dtype=ml_dtypes.bfloat16)
    maps = []
    for c in range(N_CORES):
        maps.append({
            "zf": np.roll(z, -c * BLK, axis=0),
            "eye": eye,
            "eyeb": eyeb,
        })
    return maps


def _finish(results):
    total = 0.0
    for c in range(N_CORES):
        dpos = results[c]["dpos"].astype(np.float64)
        rs = results[c]["rs"].astype(np.float64)
        total += (TEMP_INV * dpos - TEMP_INV - np.log(rs)).sum()
    return np.float32(-total / N)


def kernel(z):
    from concourse import bass_utils
    nc = _get_nc()
    res = bass_utils.run_bass_kernel_spmd(nc, _in_maps(z),
                                          core_ids=list(range(N_CORES)))
    return _finish(res.results)


# revision 11
# speedup vs baseline: 2.0423x; 1.4017x over previous
"""Contrastive loss (SimCLR-style) TRN2 Bass kernel, 8 cores, fp8, triangular.

Math: z [8192, 256] f32 ->
  zn = z / ||z||row ; S = (zn @ zn.T)/0.1 ; diag masked; row log_softmax;
  loss = -mean_i( S[i, pos(i)] - logsumexp_j S[i, j] ), pos(i) = (i+4096) % 8192.

Strategy: rows sharded 8 ways; each core's zf is ROTATED (np.roll) so its own
1024 rows sit at rotated rows/cols [0, 1024) -- one uniform SPMD program.
S is symmetric, so each core computes only a cyclic column window: with
512-row blocks and window D(i) = {i..i+8} (mod 16), its two row halves cover
rotated cols [0, 4608) and [512, 5120). Row sums over the window come from
the fused ACT Exp(10x-10) accumulate; the mirror contributions (d in 1..7)
are column sums of the exp'd tiles, accumulated across the 8 m-tiles by a
ones-vector matmul into a [1, w] PSUM row and shipped to the host, which adds
them into the right global row sums (d=0 is the masked diagonal, d=8 is
computed by both mirror cores for their own rows -- exact partition, no
double counting). Pipeline per 1024-col group: SWDGE cast-load f32->bf16,
DVE normalize (squares fused with row-reduce via scalar_tensor_tensor
accum_out; Quake rsqrt + 2 Newton steps keeps ACT's Exp table resident),
PE-transpose zn into PSUM bf16 (no DRAM bounce), DVE quantize-copy to SBUF
fp8e4, fp8 DoubleRow matmuls (K=256 per instruction). d_pos / d_ii are
diagonals of fixed PSUM sub-blocks (one fused eye-mask multiply-accumulate
each). fp8 noise averages out over rows (~1e-6 end-to-end). Host:
loss = -mean(10*d_pos - 10 - log(rowsum)).
"""

import numpy as np

N = 8192
K = 256
N_CORES = 8
BLK = N // N_CORES          # 1024 rows per core
MT = BLK // 128             # 8 m-tiles per core
TPG = 8                     # row tiles per pipeline group
GW = TPG * 128              # 1024 columns per group
NGROUP = 5                  # window [0, 5120) = 5 groups
WIN = NGROUP * GW           # 5120
POS_G = 4                   # group covering cols [4096, 5120): pos diagonal
TEMP_INV = 10.0             # 1/temperature
QMAGIC = 0x5F3759DF

_CACHE = {}


def _regions(g, half):
    """(lo, hi) of the matmul+exp region inside group g for a row half."""
    if half == 0:                       # rows [0, 512): window [0, 4608)
        return (0, 512) if g == NGROUP - 1 else (0, GW)
    else:                               # rows [512, 1024): window [512, 5120)
        return (512, GW) if g == 0 else (0, GW)


def _cs_range(g):
    """(lo, hi, mt0, mt1) colsum range + contributing m-tiles for group g."""
    if g == 0:
        return 512, GW, 0, 3            # halfA d 1..7 hits cols [512, 1024)
    if g == NGROUP - 1:
        return 0, 512, 4, 7             # halfB d 1..7 hits cols [4096, 4608)
    return 0, GW, 0, 7


def _build():
    import concourse.bass as bass
    import concourse.tile as tile
    from concourse import bacc, mybir
    from concourse.bass_interp import get_hw_module

    F32, BF16, F16 = mybir.dt.float32, mybir.dt.bfloat16, mybir.dt.float16
    FP8 = mybir.dt.float8e4
    I32 = mybir.dt.int32
    AF, ALU = mybir.ActivationFunctionType, mybir.AluOpType
    AX = mybir.AxisListType
    PM = mybir.MatmulPerfMode

    nc = bacc.Bacc("TRN2", target_bir_lowering=False, debug=False,
                   enable_asserts=False, num_devices=N_CORES)

    zf_in = nc.dram_tensor("zf", [N, K], F32, kind="ExternalInput").ap()
    eye_in = nc.dram_tensor("eye", [128, 128], F32, kind="ExternalInput").ap()
    eyeb_in = nc.dram_tensor("eyeb", [128, 128], BF16, kind="ExternalInput").ap()
    dpos_out = nc.dram_tensor("dpos", [128, MT], F32, kind="ExternalOutput").ap()
    rs_out = nc.dram_tensor("rs", [128, MT], F32, kind="ExternalOutput").ap()
    cs_out = nc.dram_tensor("cs", [1, WIN], F32, kind="ExternalOutput").ap()

    with tile.TileContext(nc) as tc:
        with (
            tc.tile_pool(name="big", bufs=1) as big,
            tc.tile_pool(name="pipe", bufs=3) as pipe,
            tc.tile_pool(name="work", bufs=2) as work,
            tc.tile_pool(name="stat", bufs=1) as stat,
            tc.tile_pool(name="ps", bufs=2, space=bass.MemorySpace.PSUM) as psp,
            tc.tile_pool(name="pt", bufs=1, space=bass.MemorySpace.PSUM) as ptp,
            tc.tile_pool(name="cs", bufs=1, space=bass.MemorySpace.PSUM) as csp,
        ):
            magic = stat.tile([128, TPG], I32)
            nc.vector.memset(magic[:], QMAGIC)
            bias_m10 = stat.tile([128, 1], F32)
            nc.vector.memset(bias_m10[:], -TEMP_INV)
            ones_b = stat.tile([128, 1], BF16)
            nc.vector.memset(ones_b[:], 1.0)
            eye = stat.tile([128, 128], F32)
            nc.sync.dma_start(eye[:], eye_in)
            eyeb = stat.tile([128, 128], BF16)
            nc.sync.dma_start(eyeb[:], eyeb_in)

            znT_own = big.tile([128, 2, BLK], FP8)   # lhsT: own cols [0, 1024)
            rs_part = stat.tile([128, MT, NGROUP], F32)
            d_pos = stat.tile([128, MT], F32)
            d_ii = stat.tile([128, MT], F32)

            def rsqrt_dve(ss, tag):
                """rsq = 1/sqrt(ss), Quake init + 2 Newton iterations (DVE)."""
                ssi = ss[:].bitcast(I32)
                sh = work.tile([128, TPG], I32, tag="sh")
                nc.vector.tensor_scalar(sh[:], ssi, 1, None,
                                        op0=ALU.arith_shift_right)
                y = stat.tile([128, TPG], F32, tag=f"y_{tag}")
                yi = y[:].bitcast(I32)
                nc.vector.tensor_sub(yi, magic[:], sh[:])
                for it in range(2):
                    y2 = work.tile([128, TPG], F32, tag="nwt")
                    nc.vector.tensor_mul(y2[:], y[:], y[:])
                    xy2 = work.tile([128, TPG], F32, tag="nwt")
                    nc.vector.tensor_mul(xy2[:], ss[:], y2[:])
                    c = work.tile([128, TPG], F32, tag="nwt")
                    nc.vector.tensor_scalar(c[:], xy2[:], -0.5, 1.5,
                                            op0=ALU.mult, op1=ALU.add)
                    yn = stat.tile([128, TPG], F32, tag=f"y{it}_{tag}")
                    nc.vector.tensor_mul(yn[:], y[:], c[:])
                    y = yn
                return y

            # ---- preload window row tiles (f32 -> bf16 cast on SWDGE) ----
            g_zbf = []
            for g in range(NGROUP):
                zt = pipe.tile([128, TPG, K], BF16, tag="zbf")
                nc.gpsimd.dma_start(
                    zt[:], zf_in[g * GW:(g + 1) * GW, :].rearrange(
                        "(t p) k -> p t k", p=128))
                g_zbf.append(zt)

            def extract_diag(ps, off, dst, mt):
                tmp = work.tile([128, 128], F32, tag="dg")
                nc.vector.scalar_tensor_tensor(
                    tmp[:], ps[:, off:off + 128], 1.0, eye[:],
                    op0=ALU.mult, op1=ALU.mult,
                    accum_out=dst[:, mt:mt + 1])

            # ---- main pipeline over column groups ----
            for g in range(NGROUP):
                zbf = g_zbf[g]
                ss = stat.tile([128, TPG], F32, tag=f"ss{g}")
                for t in range(TPG):
                    sq = work.tile([128, K], F16, tag="sq")
                    nc.vector.scalar_tensor_tensor(
                        sq[:], zbf[:, t, :], 1.0, zbf[:, t, :],
                        op0=ALU.mult, op1=ALU.mult,
                        accum_out=ss[:, t:t + 1])
                rsq = rsqrt_dve(ss, f"g{g}")
                zn = pipe.tile([128, TPG, K], BF16, tag="zn")
                for t in range(TPG):
                    nc.vector.tensor_scalar(zn[:, t, :], zbf[:, t, :],
                                            rsq[:, t:t + 1], None, op0=ALU.mult)
                # PE-transpose zn into PSUM (bf16), quantize-copy to SBUF fp8
                pt = ptp.tile([128, 2, GW], BF16, tag="pt")
                for t in range(TPG):
                    for j in range(2):
                        nc.tensor.transpose(
                            pt[:, j, t * 128:(t + 1) * 128],
                            zn[:, t, j * 128:(j + 1) * 128], eyeb[:])
                if g == 0:
                    rhs = znT_own[:]
                else:
                    znTg = pipe.tile([128, 2, GW], FP8, tag="znT8")
                    rhs = znTg[:]
                nc.vector.tensor_scalar(rhs[:, 0, :], pt[:, 0, :], 1.0, None,
                                        op0=ALU.mult)
                nc.vector.tensor_scalar(rhs[:, 1, :], pt[:, 1, :], 1.0, None,
                                        op0=ALU.mult)
                # fp8 DoubleRow matmuls + fused exp/accumulate + colsums
                cs_lo, cs_hi, cs_m0, cs_m1 = _cs_range(g)
                cs_ps = csp.tile([1, cs_hi - cs_lo], F32, tag="cs")
                for mt in range(MT):
                    lo, hi = _regions(g, mt // 4)
                    ps = psp.tile([128, GW], F32, tag="ps")
                    for s0 in range(lo, hi, 512):
                        nc.tensor.matmul(ps[:, s0:s0 + 512],
                                         znT_own[:, :, mt * 128:(mt + 1) * 128],
                                         rhs[:, :, s0:s0 + 512],
                                         start=True, stop=True,
                                         perf_mode=PM.DoubleRow)
                    if g == 0:
                        extract_diag(ps, mt * 128, d_ii, mt)
                    elif g == POS_G:
                        extract_diag(ps, mt * 128, d_pos, mt)
                    expo = work.tile([128, GW], BF16, tag="expo")
                    nc.scalar.activation(expo[:, lo:hi], ps[:, lo:hi], AF.Exp,
                                         bias=bias_m10[:], scale=TEMP_INV,
                                         accum_out=rs_part[:, mt, g:g + 1])
                    if cs_m0 <= mt <= cs_m1:
                        for s0 in range(cs_lo, cs_hi, 512):
                            nc.tensor.matmul(
                                cs_ps[:, s0 - cs_lo:s0 - cs_lo + 512],
                                ones_b[:], expo[:, s0:s0 + 512],
                                start=(mt == cs_m0), stop=(mt == cs_m1),
                                skip_group_check=True)
                cs_sb = work.tile([1, cs_hi - cs_lo], F32, tag="cssb")
                nc.vector.tensor_copy(cs_sb[:], cs_ps[:])
                nc.sync.dma_start(cs_out[0:1, g * GW + cs_lo:g * GW + cs_hi],
                                  cs_sb[:])

            # ---- epilogue: own-window rowsum minus self term ----
            rs_sum = stat.tile([128, MT], F32)
            nc.vector.reduce_sum(rs_sum[:], rs_part[:], axis=AX.X)
            self_t = stat.tile([128, MT], F32)
            nc.scalar.activation(self_t[:], d_ii[:], AF.Exp,
                                 bias=bias_m10[:], scale=TEMP_INV)
            rs_corr = stat.tile([128, MT], F32)
            nc.vector.tensor_sub(rs_corr[:], rs_sum[:], self_t[:])

            nc.sync.dma_start(dpos_out, d_pos[:])
            nc.sync.dma_start(rs_out, rs_corr[:])

    nc.compile()
    nc.m = get_hw_module(nc.m)
    return nc


def _get_nc():
    if "nc" not in _CACHE:
        _CACHE["nc"] = _build()
    return _CACHE["nc"]


def _in_maps(z):
    import ml_dtypes
    z = np.ascontiguousarray(z, dtype=np.float32)
    eye = np.eye(128, dtype=np.float32)
    eyeb = np.eye(128, dtype=ml_dtypes.bfloat16)
    maps = []
    for c in range(N_CORES):
        maps.append({
            "zf": np.roll(z, -c * BLK, axis=0),
            "eye": eye,
            "eyeb": eyeb,
        })
    return maps


def _finish(results):
    rowsum = np.zeros(N, dtype=np.float64)
    dpos_g = np.zeros(N, dtype=np.float64)
    for c in range(N_CORES):
        # own-window rowsum (self already subtracted); layout [p, mt]
        rowsum[c * BLK:(c + 1) * BLK] += \
            results[c]["rs"].astype(np.float64).T.reshape(-1)
        dpos_g[c * BLK:(c + 1) * BLK] = \
            results[c]["dpos"].astype(np.float64).T.reshape(-1)
        # mirror contributions: rotated col j -> global row (c*BLK + j) % N
        # (device writes only [512, 4608); edges stay unused)
        full = np.zeros(N, dtype=np.float64)
        full[512:WIN - 512] = results[c]["cs"].astype(np.float64)[0, 512:WIN - 512]
        rowsum += np.roll(full, c * BLK)
    total = (TEMP_INV * dpos_g - TEMP_INV - np.log(rowsum)).sum()
    return np.float32(-total / N)


def kernel(z):
    from concourse import bass_utils
    nc = _get_nc()
    res = bass_utils.run_bass_kernel_spmd(nc, _in_maps(z),
                                          core_ids=list(range(N_CORES)))
    return _finish(res.results)


# revision 15
# speedup vs baseline: 2.0978x; 1.0271x over previous
"""Contrastive loss (SimCLR-style) TRN2 Bass kernel, 8 cores, fp8, triangular.

Math: z [8192, 256] f32 ->
  zn = z / ||z||row ; S = (zn @ zn.T)/0.1 ; diag masked; row log_softmax;
  loss = -mean_i( S[i, pos(i)] - logsumexp_j S[i, j] ), pos(i) = (i+4096) % 8192.

Strategy: rows sharded 8 ways; each core's zf is ROTATED (np.roll) so its own
1024 rows sit at rotated rows/cols [0, 1024) -- one uniform SPMD program.
S is symmetric, so each core computes only a cyclic column window: with
512-row blocks and window D(i) = {i..i+8} (mod 16), its two row halves cover
rotated cols [0, 4608) and [512, 5120). Row sums over the window come from
the fused ACT Exp(10x-10) accumulate; the mirror contributions (d in 1..7)
are column sums of the exp'd tiles, accumulated across the 8 m-tiles by a
ones-vector matmul into a [1, w] PSUM row and shipped to the host, which adds
them into the right global row sums (d=0 is the masked diagonal, d=8 is
computed by both mirror cores for their own rows -- exact partition, no
double counting). Pipeline per 1024-col group: SWDGE cast-load f32->bf16,
DVE normalize (squares fused with row-reduce via scalar_tensor_tensor
accum_out; Quake rsqrt + 2 Newton steps keeps ACT's Exp table resident),
PE-transpose zn into PSUM bf16 (no DRAM bounce), DVE quantize-copy to SBUF
fp8e4, fp8 DoubleRow matmuls (K=256 per instruction). d_pos / d_ii are
diagonals of fixed PSUM sub-blocks (one fused eye-mask multiply-accumulate
each). fp8 noise averages out over rows (~1e-6 end-to-end). Host:
loss = -mean(10*d_pos - 10 - log(rowsum)).
"""

import numpy as np

N = 8192
K = 256
N_CORES = 8
BLK = N // N_CORES          # 1024 rows per core
MT = BLK // 128             # 8 m-tiles per core
TPG = 8                     # row tiles per pipeline group
GW = TPG * 128              # 1024 columns per group
NGROUP = 5                  # window [0, 5120) = 5 groups
WIN = NGROUP * GW           # 5120
POS_G = 4                   # group covering cols [4096, 5120): pos diagonal
TEMP_INV = 10.0             # 1/temperature
QMAGIC = 0x5F3759DF

_CACHE = {}


def _regions(g, half):
    """(lo, hi) of the matmul+exp region inside group g for a row half."""
    if half == 0:                       # rows [0, 512): window [0, 4608)
        return (0, 512) if g == NGROUP - 1 else (0, GW)
    else:                               # rows [512, 1024): window [512, 5120)
        return (512, GW) if g == 0 else (0, GW)


def _cs_range(g):
    """(lo, hi, mt0, mt1) colsum range + contributing m-tiles for group g."""
    if g == 0:
        return 512, GW, 0, 3            # halfA d 1..7 hits cols [512, 1024)
    if g == NGROUP - 1:
        return 0, 512, 4, 7             # halfB d 1..7 hits cols [4096, 4608)
    return 0, GW, 0, 7


def _build():
    import concourse.bass as bass
    import concourse.tile as tile
    from concourse import bacc, mybir
    from concourse.bass_interp import get_hw_module

    F32, BF16, F16 = mybir.dt.float32, mybir.dt.bfloat16, mybir.dt.float16
    FP8 = mybir.dt.float8e4
    I32 = mybir.dt.int32
    AF, ALU = mybir.ActivationFunctionType, mybir.AluOpType
    AX = mybir.AxisListType
    PM = mybir.MatmulPerfMode

    nc = bacc.Bacc("TRN2", target_bir_lowering=False, debug=False,
                   enable_asserts=False, num_devices=N_CORES)

    zf_in = nc.dram_tensor("zf", [N, K], F32, kind="ExternalInput").ap()
    eye_in = nc.dram_tensor("eye", [128, 128], F32, kind="ExternalInput").ap()
    eyeb_in = nc.dram_tensor("eyeb", [128, 128], BF16, kind="ExternalInput").ap()
    dpos_out = nc.dram_tensor("dpos", [128, MT], F32, kind="ExternalOutput").ap()
    rs_out = nc.dram_tensor("rs", [128, MT], F32, kind="ExternalOutput").ap()
    cs_out = nc.dram_tensor("cs", [1, WIN], F32, kind="ExternalOutput").ap()

    with tile.TileContext(nc) as tc:
        with (
            tc.tile_pool(name="big", bufs=1) as big,
            tc.tile_pool(name="pipe", bufs=3) as pipe,
            tc.tile_pool(name="work", bufs=2) as work,
            tc.tile_pool(name="stat", bufs=1) as stat,
            tc.tile_pool(name="ps", bufs=2, space=bass.MemorySpace.PSUM) as psp,
            tc.tile_pool(name="pt", bufs=1, space=bass.MemorySpace.PSUM) as ptp,
            tc.tile_pool(name="cs", bufs=1, space=bass.MemorySpace.PSUM) as csp,
        ):
            magic = stat.tile([128, TPG], I32)
            nc.vector.memset(magic[:], QMAGIC)
            bias_m10 = stat.tile([128, 1], F32)
            nc.vector.memset(bias_m10[:], -TEMP_INV)
            ones_b = stat.tile([128, 1], BF16)
            nc.vector.memset(ones_b[:], 1.0)
            eye = stat.tile([128, 128], F32)
            nc.sync.dma_start(eye[:], eye_in)
            eyeb = stat.tile([128, 128], BF16)
            nc.sync.dma_start(eyeb[:], eyeb_in)

            znT_own = big.tile([128, 2, BLK], FP8)   # lhsT: own cols [0, 1024)
            rs_part = stat.tile([128, MT, NGROUP], F32)
            d_pos = stat.tile([128, MT], F32)
            d_ii = stat.tile([128, MT], F32)

            def rsqrt_dve(ss, tag):
                """rsq = 1/sqrt(ss), Quake init + 2 Newton iterations (DVE)."""
                ssi = ss[:].bitcast(I32)
                sh = work.tile([128, TPG], I32, tag="sh")
                nc.vector.tensor_scalar(sh[:], ssi, 1, None,
                                        op0=ALU.arith_shift_right)
                y = stat.tile([128, TPG], F32, tag=f"y_{tag}")
                yi = y[:].bitcast(I32)
                nc.vector.tensor_sub(yi, magic[:], sh[:])
                for it in range(2):
                    y2 = work.tile([128, TPG], F32, tag="nwt")
                    nc.vector.tensor_mul(y2[:], y[:], y[:])
                    xy2 = work.tile([128, TPG], F32, tag="nwt")
                    nc.vector.tensor_mul(xy2[:], ss[:], y2[:])
                    c = work.tile([128, TPG], F32, tag="nwt")
                    nc.vector.tensor_scalar(c[:], xy2[:], -0.5, 1.5,
                                            op0=ALU.mult, op1=ALU.add)
                    yn = stat.tile([128, TPG], F32, tag=f"y{it}_{tag}")
                    nc.vector.tensor_mul(yn[:], y[:], c[:])
                    y = yn
                return y

            # ---- preload window row tiles (f32 -> bf16 cast on SWDGE) ----
            g_zbf = []
            for g in range(NGROUP):
                zt = pipe.tile([128, TPG, K], BF16, tag="zbf")
                h = TPG // 2
                for p0 in (0, h):
                    nc.gpsimd.dma_start(
                        zt[:, p0:p0 + h, :],
                        zf_in[g * GW + p0 * 128:g * GW + (p0 + h) * 128, :]
                        .rearrange("(t p) k -> p t k", p=128))
                g_zbf.append(zt)

            def extract_diag(ps, off, dst, mt):
                tmp = work.tile([128, 128], F32, tag="dg")
                nc.vector.scalar_tensor_tensor(
                    tmp[:], ps[:, off:off + 128], 1.0, eye[:],
                    op0=ALU.mult, op1=ALU.mult,
                    accum_out=dst[:, mt:mt + 1])

            # ---- main pipeline over column groups ----
            for g in range(NGROUP):
                zbf = g_zbf[g]
                ss = stat.tile([128, TPG], F32, tag=f"ss{g}")
                for t in range(TPG):
                    sq = work.tile([128, K], F16, tag="sq")
                    nc.vector.scalar_tensor_tensor(
                        sq[:], zbf[:, t, :], 1.0, zbf[:, t, :],
                        op0=ALU.mult, op1=ALU.mult,
                        accum_out=ss[:, t:t + 1])
                rsq = rsqrt_dve(ss, f"g{g}")
                zn = pipe.tile([128, TPG, K], BF16, tag="zn")
                for t in range(TPG):
                    nc.vector.tensor_scalar(zn[:, t, :], zbf[:, t, :],
                                            rsq[:, t:t + 1], None, op0=ALU.mult)
                # PE-transpose zn into PSUM (bf16), quantize-copy to SBUF fp8
                pt = ptp.tile([128, 2, GW], BF16, tag="pt")
                for t in range(TPG):
                    for j in range(2):
                        nc.tensor.transpose(
                            pt[:, j, t * 128:(t + 1) * 128],
                            zn[:, t, j * 128:(j + 1) * 128], eyeb[:])
                if g == 0:
                    rhs = znT_own[:]
                else:
                    znTg = pipe.tile([128, 2, GW], FP8, tag="znT8")
                    rhs = znTg[:]
                nc.vector.tensor_scalar(rhs[:, :, 0:512], pt[:, :, 0:512],
                                        1.0, None, op0=ALU.mult)
                nc.vector.tensor_scalar(rhs[:, :, 512:GW], pt[:, :, 512:GW],
                                        1.0, None, op0=ALU.mult)
                # fp8 DoubleRow matmuls + fused exp/accumulate + colsums
                cs_lo, cs_hi, cs_m0, cs_m1 = _cs_range(g)
                cs_ps = csp.tile([1, cs_hi - cs_lo], F32, tag="cs")
                for mt in range(MT):
                    lo, hi = _regions(g, mt // 4)
                    ps = psp.tile([128, GW], F32, tag="ps")
                    for s0 in range(lo, hi, 512):
                        nc.tensor.matmul(ps[:, s0:s0 + 512],
                                         znT_own[:, :, mt * 128:(mt + 1) * 128],
                                         rhs[:, :, s0:s0 + 512],
                                         start=True, stop=True,
                                         perf_mode=PM.DoubleRow)
                    if g == 0:
                        extract_diag(ps, mt * 128, d_ii, mt)
                    elif g == POS_G:
                        extract_diag(ps, mt * 128, d_pos, mt)
                    expo = work.tile([128, GW], BF16, tag="expo")
                    nc.scalar.activation(expo[:, lo:hi], ps[:, lo:hi], AF.Exp,
                                         bias=bias_m10[:], scale=TEMP_INV,
                                         accum_out=rs_part[:, mt, g:g + 1])
                    if cs_m0 <= mt <= cs_m1:
                        for s0 in range(cs_lo, cs_hi, 512):
                            nc.tensor.matmul(
                                cs_ps[:, s0 - cs_lo:s0 - cs_lo + 512],
                                ones_b[:], expo[:, s0:s0 + 512],
                                start=(mt == cs_m0), stop=(mt == cs_m1),
                                skip_group_check=True)
                cs_sb = work.tile([1, cs_hi - cs_lo], F32, tag="cssb")
                nc.vector.tensor_copy(cs_sb[:], cs_ps[:])
                nc.sync.dma_start(cs_out[0:1, g * GW + cs_lo:g * GW + cs_hi],
                                  cs_sb[:])

            # ---- epilogue: own-window rowsum minus self term ----
            rs_sum = stat.tile([128, MT], F32)
            nc.vector.reduce_sum(rs_sum[:], rs_part[:], axis=AX.X)
            self_t = stat.tile([128, MT], F32)
            nc.scalar.activation(self_t[:], d_ii[:], AF.Exp,
                                 bias=bias_m10[:], scale=TEMP_INV)
            rs_corr = stat.tile([128, MT], F32)
            nc.vector.tensor_sub(rs_corr[:], rs_sum[:], self_t[:])

            nc.sync.dma_start(dpos_out, d_pos[:])
            nc.sync.dma_start(rs_out, rs_corr[:])

    nc.compile()
    nc.m = get_hw_module(nc.m)
    return nc


def _get_nc():
    if "nc" not in _CACHE:
        _CACHE["nc"] = _build()
    return _CACHE["nc"]


def _in_maps(z):
    import ml_dtypes
    z = np.ascontiguousarray(z, dtype=np.float32)
    eye = np.eye(128, dtype=np.float32)
    eyeb = np.eye(128, dtype=ml_dtypes.bfloat16)
    maps = []
    for c in range(N_CORES):
        maps.append({
            "zf": np.roll(z, -c * BLK, axis=0),
            "eye": eye,
            "eyeb": eyeb,
        })
    return maps


def _finish(results):
    rowsum = np.zeros(N, dtype=np.float64)
    dpos_g = np.zeros(N, dtype=np.float64)
    for c in range(N_CORES):
        # own-window rowsum (self already subtracted); layout [p, mt]
        rowsum[c * BLK:(c + 1) * BLK] += \
            results[c]["rs"].astype(np.float64).T.reshape(-1)
        dpos_g[c * BLK:(c + 1) * BLK] = \
            results[c]["dpos"].astype(np.float64).T.reshape(-1)
        # mirror contributions: rotated col j -> global row (c*BLK + j) % N
        # (device writes only [512, 4608); edges stay unused)
        full = np.zeros(N, dtype=np.float64)
        full[512:WIN - 512] = results[c]["cs"].astype(np.float64)[0, 512:WIN - 512]
        rowsum += np.roll(full, c * BLK)
    total = (TEMP_INV * dpos_g - TEMP_INV - np.log(rowsum)).sum()
    return np.float32(-total / N)


def kernel(z):
    from concourse import bass_utils
    nc = _get_nc()
    res = bass_utils.run_bass_kernel_spmd(nc, _in_maps(z),
                                          core_ids=list(range(N_CORES)))
    return _finish(res.results)
